# revision 12
# baseline (speedup 1.0000x reference)
"""Distributed Trainium2 (Bass) kernel for nn_Attention_53764400611491.

The reference module has HEADS == C == 64, so head_dim d = C//HEADS = 1.
With d = 1 the attention algebra collapses: per (batch b, head c)

    attn = q k^T            (outer product, [N,N])
    o    = attn @ v  =  q * (k . v)        <- a scalar per (b,c)!

so the whole module is

    out[b,c,n] = sum_c' wp[c,c'] * q[b,c',n] * s[b,c'] + x[b,c,n]
    q = wq @ x_b          s[b,c'] = sum_n (wk@x_b)[c',n] * (wv@x_b)[c',n]

and the [b,h,N,N] attention tensor never needs to exist.

Sharding over 8 NeuronCores: core i handles batch b = i//4 and output
n-chunk j = i%4 (256 of the 1024 flattened h*w positions).  Each core
receives the full x_b (rotated so its own chunk comes first), computes
s_b redundantly, and writes its 64x256 output chunk.  No collectives:
an 8-core AllReduce has a ~10us latency floor.

v6 data path (fp16 on the wire, f32 accumulation in PSUM):
  - x ships fp16 in two [64,512] halves on the two HWDGE rings
    (SP + ACT) into one SBUF tensor; packed weights [k|v|wq^T|wp^T]
    fp16 [64,256] go on the gpsimd SWDGE ring.  One dma_start per
    ring: each dynamic DMA pays ~1.5us fixed latency + stripe
    straggle, so fewer is faster.
  - One stacked matmul per x half computes [k; v] = [wk.T|wv.T] @ x
    into PSUM (k on partitions 0-63, v on 64-127).  ACT evicts the v
    rows to SBUF fp16; DVE then computes s[c] = sum_n k*v in a single
    fused scalar_tensor_tensor (k from PSUM, v from SBUF, accum_out =
    row sum), one per half.  No square table, no ACT accumulator
    read-out, no u/d subtract.
  - wpTs = wp^T * s feeds the final fp16 matmul; the residual "+x" is
    folded into the PSUM->SBUF eviction as a DVE tensor_tensor add.
    The final matmul / eviction / output DMA are split into two
    128-column halves so the first half's DMA (on SP) issues while
    the second half (on ACT) is still being computed.
  - There is NO nc.Block(): engine streams are emitted straight into
    the main body, so bass adds no end-of-body all-engine barrier and
    the compiler's own teardown epilogue starts as soon as the last
    engine's body ends.  The output DMA's completion is not waited on:
    the multi-microsecond teardown (per-engine semaphore clears)
    covers the transfer, and nothing in the kernel consumes out_sem.
Measured end-to-end relative error ~1e-3 (threshold 2e-2).
"""
import numpy as np

import concourse.bass as bass
import concourse.mybir as mybir
from concourse.bass_utils import run_bass_kernel_spmd

F32 = mybir.dt.float32
F16 = mybir.dt.float16
MULT = mybir.AluOpType.mult
ADD = mybir.AluOpType.add
COPY = mybir.ActivationFunctionType.Copy

B, C, H, W = 2, 64, 32, 32
N = H * W          # 1024
NCHUNK = N // 4    # 256 output columns per core


def _build_nc() -> bass.Bass:
    nc = bass.Bass()
    xlo_ext = nc.declare_dram_parameter("xlo", [64, 512], F16, isOutput=False)
    xhi_ext = nc.declare_dram_parameter("xhi", [64, 512], F16, isOutput=False)
    w_ext = nc.declare_dram_parameter("w", [64, 256], F16, isOutput=False)
    o_ext = nc.declare_dram_parameter("out", [64, 256], F16, isOutput=True)

    from contextlib import ExitStack

    with ExitStack() as ctx:
        e = ctx.enter_context
        X = e(nc.sbuf_tensor("X", [64, 1024], F16))
        Wsb = e(nc.sbuf_tensor("Wsb", [64, 256], F16))   # [kv | wqT | wpT]
        vsb = e(nc.sbuf_tensor("vsb", [128, 1024], F16))  # v rows on partitions 64-127
        sqb = e(nc.sbuf_tensor("sqb", [64, 1024], F16))   # k*v scratch
        redc = e(nc.sbuf_tensor("redc", [64, 2], F32))    # per-half row sums of k*v
        s4 = e(nc.sbuf_tensor("s4", [64, 1], F32))
        Qsb = e(nc.sbuf_tensor("Qsb", [64, 256], F16))
        wpTs = e(nc.sbuf_tensor("wpTs", [64, 64], F16))
        Fsb = e(nc.sbuf_tensor("Fsb", [64, 256], F16))
        uv1 = e(nc.psum_tensor("uv1", [128, 512], F32))
        uv2 = e(nc.psum_tensor("uv2", [128, 512], F32))
        Qp = e(nc.psum_tensor("Qp", [64, 256], F32))
        Op = e(nc.psum_tensor("Op", [64, 256], F32))
        xlo_sem = e(nc.semaphore("xlo_sem"))
        xhi_sem = e(nc.semaphore("xhi_sem"))
        w_sem = e(nc.semaphore("w_sem"))
        pe_sem = e(nc.semaphore("pe_sem"))
        act_sem = e(nc.semaphore("act_sem"))
        dv_sem = e(nc.semaphore("dv_sem"))
        out_sem = e(nc.semaphore("out_sem"))

        kv = Wsb[:, 0:128]
        wqT = Wsb[:, 128:192]
        wpT = Wsb[:, 192:256]

        sync, gp, pe, act, dv = nc.sync, nc.gpsimd, nc.tensor, nc.scalar, nc.vector

        # ---- SP (sync): x low half in, first result half out ----
        sync.dma_start(X[:, 0:512], xlo_ext[:]).then_inc(xlo_sem, 16)
        sync.wait_ge(dv_sem, 4)
        # completion covered by the NEFF teardown epilogue (see header)
        sync.dma_start(o_ext[:, 0:128], Fsb[:, 0:128]).then_inc(out_sem, 16)

        # ---- GpSimd: weights in ----
        gp.dma_start(Wsb[:], w_ext[:]).then_inc(w_sem, 16)

        # ---- PE ----
        pe.wait_ge(w_sem, 16)
        pe.wait_ge(xlo_sem, 16)
        # rows 0-63 = k = wk@x, rows 64-127 = v = wv@x
        pe.matmul(uv1[:], kv, X[:, 0:512], start=True, stop=True).then_inc(pe_sem, 1)
        pe.wait_ge(xhi_sem, 16)
        pe.matmul(uv2[:], kv, X[:, 512:1024], start=True, stop=True).then_inc(pe_sem, 1)
        pe.matmul(Qp[:], wqT, X[:, 0:256], start=True, stop=True).then_inc(pe_sem, 1)
        # out = (wp diag(s)) @ q in two column halves ("+x" added on eviction)
        pe.wait_ge(dv_sem, 3)
        pe.wait_ge(act_sem, 3)
        pe.matmul(Op[:, 0:128], wpTs[:], Qsb[:, 0:128], start=True, stop=True).then_inc(pe_sem, 1)
        pe.matmul(Op[:, 128:256], wpTs[:], Qsb[:, 128:256], start=True, stop=True).then_inc(pe_sem, 1)

        # ---- ACT (scalar): x high half in, v eviction, second out half ----
        act.dma_start(X[:, 512:1024], xhi_ext[:]).then_inc(xhi_sem, 16)
        act.wait_ge(pe_sem, 1)
        act.activation(vsb[64:128, 0:512], uv1[64:128, :], COPY).then_inc(act_sem, 1)
        act.wait_ge(pe_sem, 2)
        act.activation(vsb[64:128, 512:1024], uv2[64:128, :], COPY).then_inc(act_sem, 1)
        # q PSUM -> SBUF fp16 for the final matmul
        act.wait_ge(pe_sem, 3)
        act.activation(Qsb[:], Qp[:], COPY).then_inc(act_sem, 1)
        act.wait_ge(dv_sem, 5)
        act.dma_start(o_ext[:, 128:256], Fsb[:, 128:256]).then_inc(out_sem, 16)

        # ---- DVE (vector): fused k*v row-sums, wpTs, residual eviction ----
        dv.wait_ge(act_sem, 1)
        dv.scalar_tensor_tensor(sqb[:, 0:512], uv1[0:64, :], 1.0, vsb[64:128, 0:512],
                                MULT, MULT, accum_out=redc[:, 0:1]).then_inc(dv_sem, 1)
        dv.wait_ge(act_sem, 2)
        dv.scalar_tensor_tensor(sqb[:, 512:1024], uv2[0:64, :], 1.0, vsb[64:128, 512:1024],
                                MULT, MULT, accum_out=redc[:, 1:2]).then_inc(dv_sem, 1)
        dv.drain()  # redc landed (same-engine RAW)
        dv.reduce_sum(s4[:], redc[:], axis=mybir.AxisListType.X)
        dv.drain()  # s4 landed
        dv.tensor_scalar(wpTs[:], wpT, s4[:], None, op0=MULT).then_inc(dv_sem, 1)
        dv.wait_ge(pe_sem, 4)
        dv.tensor_tensor(Fsb[:, 0:128], Op[:, 0:128], X[:, 0:128], ADD).then_inc(dv_sem, 1)
        dv.wait_ge(pe_sem, 5)
        dv.tensor_tensor(Fsb[:, 128:256], Op[:, 128:256], X[:, 128:256], ADD).then_inc(dv_sem, 1)

    return nc


def _shard_inputs(x, wq, wk, wv, wp):
    """Full inputs -> list of 8 per-core {'xlo','xhi','w'} dicts (fp16)."""
    x = np.asarray(x, dtype=np.float32)
    wq, wk, wv, wp = (np.asarray(a, dtype=np.float32) for a in (wq, wk, wv, wp))
    xf = x.reshape(B, C, N)
    kv = np.concatenate([wk.T, wv.T], axis=1)                          # [64,128]
    w = np.concatenate([kv, wq.T, wp.T], axis=1).astype(np.float16)    # [64,256]
    w = np.ascontiguousarray(w)
    in_maps = []
    for core in range(8):
        bb, j = core // 4, core % 4
        xr = np.roll(xf[bb], -j * NCHUNK, axis=1).astype(np.float16)   # [64,1024]
        in_maps.append({
            "xlo": np.ascontiguousarray(xr[:, 0:512]),
            "xhi": np.ascontiguousarray(xr[:, 512:1024]),
            "w": w,
        })
    return in_maps


def _gather_outputs(results):
    """8 per-core {'out': [64,256] fp16} -> full [b,C,h,w] f32."""
    out = np.empty((B, C, N), dtype=np.float32)
    for core in range(8):
        bb, j = core // 4, core % 4
        out[bb, :, j * NCHUNK:(j + 1) * NCHUNK] = np.asarray(results[core]["out"]).astype(np.float32)
    return out.reshape(B, C, H, W)


_NC_CACHE = None


def kernel(x, wq, wk, wv, wp) -> np.ndarray:
    global _NC_CACHE
    if _NC_CACHE is None:
        _NC_CACHE = _build_nc()
    in_maps = _shard_inputs(x, wq, wk, wv, wp)
    last_err = None
    for _ in range(3):
        try:
            res = run_bass_kernel_spmd(_NC_CACHE, in_maps, core_ids=list(range(8)))
            return _gather_outputs(res.results)
        except Exception as exc:  # transient device-unrecoverable resets on retry
            last_err = exc
    raise last_err


# revision 15
# speedup vs baseline: 1.2479x; 1.2479x over previous
"""Distributed Trainium2 (Bass) kernel for nn_Attention_53764400611491.

The reference module has HEADS == C == 64, so head_dim d = C//HEADS = 1.
With d = 1 the attention algebra collapses: per (batch b, head c)

    attn = q k^T            (outer product, [N,N])
    o    = attn @ v  =  q * (k . v)        <- a scalar per (b,c)!

so the whole module is

    out[b,c,n] = sum_c' wp[c,c'] * q[b,c',n] * s[b,c'] + x[b,c,n]
    q = wq @ x_b          s[b,c'] = sum_n (wk@x_b)[c',n] * (wv@x_b)[c',n]

and the [b,h,N,N] attention tensor never needs to exist.  With
u = (wk+wv) @ x and d = (wk-wv) @ x:   s = (sum u^2 - sum d^2) / 4.

Sharding over 8 NeuronCores: core i handles batch b = i//4 and output
n-chunk j = i%4 (256 of the 1024 flattened h*w positions).  Each core
receives the full x_b (rotated so its own chunk comes first), computes
s_b redundantly, and writes its 64x256 output chunk.  No collectives:
an 8-core AllReduce has a ~10us latency floor.

v7 data path (fp16 on the wire, f32 accumulation in PSUM):
  - x ships fp16 in two [64,512] halves on the two HWDGE rings
    (SP + ACT) into one SBUF tensor; packed weights [kv|wq^T|wp^T]
    fp16 [64,256] go on the gpsimd SWDGE ring.  One dma_start per
    ring: each dynamic DMA pays ~1.5us fixed latency + stripe
    straggle, so fewer is faster.
  - uv = Wkv @ x runs as two stacked fp16 matmuls (one per half); ACT
    squares each half straight out of PSUM with accum_out=, which
    fuses the row-reduction into the same pass (no DVE reduce over
    [128,1024]).  A dummy Square issued before any waits prefetches
    the ACT PWP table off the critical path.
  - wpTs = wp^T * s4 * 0.25 on DVE feeds the final fp16 matmul; the
    residual "+x" is folded into the PSUM->SBUF eviction as a DVE
    tensor_tensor add.  The final matmul / eviction / output DMA are
    split into two 128-column halves so the first half's DMA (on SP)
    issues while the second half (on ACT) is still being computed.
  - There is NO nc.Block(): engine streams are emitted straight into
    the main body, so bass adds no end-of-body all-engine barrier and
    the compiler's own teardown epilogue (which begins with its own
    all-engine sync) starts as soon as the last engine's body ends.
    The output DMAs' completion is not waited on anywhere: the
    multi-microsecond teardown (per-engine semaphore clears) covers
    the transfers, and nothing in the kernel consumes out_sem.
Measured end-to-end relative error ~8e-4 (threshold 2e-2).
"""
import numpy as np

import concourse.bass as bass
import concourse.mybir as mybir
from concourse.bass_utils import run_bass_kernel_spmd

F32 = mybir.dt.float32
F16 = mybir.dt.float16
MULT = mybir.AluOpType.mult
SUB = mybir.AluOpType.subtract
ADD = mybir.AluOpType.add
SQUARE = mybir.ActivationFunctionType.Square
COPY = mybir.ActivationFunctionType.Copy

B, C, H, W = 2, 64, 32, 32
N = H * W          # 1024
NCHUNK = N // 4    # 256 output columns per core


def _build_nc() -> bass.Bass:
    nc = bass.Bass()
    xlo_ext = nc.declare_dram_parameter("xlo", [64, 512], F16, isOutput=False)
    xhi_ext = nc.declare_dram_parameter("xhi", [64, 512], F16, isOutput=False)
    w_ext = nc.declare_dram_parameter("w", [64, 256], F16, isOutput=False)
    o_ext = nc.declare_dram_parameter("out", [64, 256], F16, isOutput=True)

    from contextlib import ExitStack

    with ExitStack() as ctx:
        e = ctx.enter_context
        X = e(nc.sbuf_tensor("X", [64, 1024], F16))
        Wsb = e(nc.sbuf_tensor("Wsb", [64, 256], F16))   # [kv | wqT | wpT]
        sqb = e(nc.sbuf_tensor("sqb", [128, 1024], F32))  # square scratch
        redc = e(nc.sbuf_tensor("redc", [128, 2], F32))   # per-half row sums
        redall = e(nc.sbuf_tensor("redall", [128, 1], F32))
        s4 = e(nc.sbuf_tensor("s4", [64, 1], F32))
        Qsb = e(nc.sbuf_tensor("Qsb", [64, 256], F16))
        wpTs = e(nc.sbuf_tensor("wpTs", [64, 64], F16))
        Fsb = e(nc.sbuf_tensor("Fsb", [64, 256], F16))
        dummy = e(nc.sbuf_tensor("warmup", [1, 1], F32))
        uv1 = e(nc.psum_tensor("uv1", [128, 512], F32))
        uv2 = e(nc.psum_tensor("uv2", [128, 512], F32))
        Qp = e(nc.psum_tensor("Qp", [64, 256], F32))
        Op = e(nc.psum_tensor("Op", [64, 256], F32))
        xlo_sem = e(nc.semaphore("xlo_sem"))
        xhi_sem = e(nc.semaphore("xhi_sem"))
        w_sem = e(nc.semaphore("w_sem"))
        pe_sem = e(nc.semaphore("pe_sem"))
        act_sem = e(nc.semaphore("act_sem"))
        dv_sem = e(nc.semaphore("dv_sem"))
        out_sem = e(nc.semaphore("out_sem"))

        kv = Wsb[:, 0:128]
        wqT = Wsb[:, 128:192]
        wpT = Wsb[:, 192:256]

        sync, gp, pe, act, dv = nc.sync, nc.gpsimd, nc.tensor, nc.scalar, nc.vector

        # ---- SP (sync): x low half in, first result half out ----
        sync.dma_start(X[:, 0:512], xlo_ext[:]).then_inc(xlo_sem, 16)
        sync.wait_ge(dv_sem, 3)
        # completion covered by the NEFF teardown epilogue (see header)
        sync.dma_start(o_ext[:, 0:128], Fsb[:, 0:128]).then_inc(out_sem, 16)

        # ---- GpSimd: weights in ----
        gp.dma_start(Wsb[:], w_ext[:]).then_inc(w_sem, 16)

        # ---- PE ----
        pe.wait_ge(w_sem, 16)
        pe.wait_ge(xlo_sem, 16)
        # rows 0-63 = u = (wk+wv)x, rows 64-127 = d = (wk-wv)x
        pe.matmul(uv1[:], kv, X[:, 0:512], start=True, stop=True).then_inc(pe_sem, 1)
        pe.wait_ge(xhi_sem, 16)
        pe.matmul(uv2[:], kv, X[:, 512:1024], start=True, stop=True).then_inc(pe_sem, 1)
        pe.matmul(Qp[:], wqT, X[:, 0:256], start=True, stop=True).then_inc(pe_sem, 1)
        # out = (wp diag(s)) @ q in two column halves ("+x" added on eviction)
        pe.wait_ge(dv_sem, 1)
        pe.wait_ge(act_sem, 4)
        pe.matmul(Op[:, 0:128], wpTs[:], Qsb[:, 0:128], start=True, stop=True).then_inc(pe_sem, 1)
        pe.matmul(Op[:, 128:256], wpTs[:], Qsb[:, 128:256], start=True, stop=True).then_inc(pe_sem, 1)

        # ---- ACT (scalar): x high half in, fused squares, second out half ----
        act.dma_start(X[:, 512:1024], xhi_ext[:]).then_inc(xhi_sem, 16)
        # warm the ACT Square table while DMAs are in flight
        act.activation(dummy[:], nc.const_aps.tensor(0.0, (1, 1), F32), SQUARE).then_inc(act_sem, 1)
        act.wait_ge(pe_sem, 1)
        act.activation(sqb[:, 0:512], uv1[:], SQUARE, accum_out=redc[:, 0:1]).then_inc(act_sem, 1)
        act.wait_ge(pe_sem, 2)
        act.activation(sqb[:, 512:1024], uv2[:], SQUARE, accum_out=redc[:, 1:2]).then_inc(act_sem, 1)
        # q PSUM -> SBUF fp16 for the final matmul
        act.wait_ge(pe_sem, 3)
        act.activation(Qsb[:], Qp[:], COPY).then_inc(act_sem, 1)
        act.wait_ge(dv_sem, 4)
        act.dma_start(o_ext[:, 128:256], Fsb[:, 128:256]).then_inc(out_sem, 16)

        # ---- DVE (vector) ----
        dv.wait_ge(act_sem, 3)
        dv.reduce_sum(redall[:], redc[:], axis=mybir.AxisListType.X)
        dv.drain()  # redall landed (same-engine RAW)
        # s4 = sum u^2 - sum d^2  (cross-base scalar operand)  = 4*s
        dv.tensor_scalar(s4[:], redall[0:64, :], redall[64:128, :], None, op0=SUB)
        dv.drain()  # s4 landed
        # wpTs = wp.T * s4 * 0.25  (fold the /4 of the +- identity)
        dv.tensor_scalar(wpTs[:], wpT, s4[:], 0.25, op0=MULT, op1=MULT).then_inc(dv_sem, 1)
        dv.wait_ge(pe_sem, 4)
        # evict out halves PSUM -> SBUF with the "+ x" residual folded in
        dv.tensor_tensor(Fsb[:, 0:128], Op[:, 0:128], X[:, 0:128], ADD).then_inc(dv_sem, 2)
        dv.wait_ge(pe_sem, 5)
        dv.tensor_tensor(Fsb[:, 128:256], Op[:, 128:256], X[:, 128:256], ADD).then_inc(dv_sem, 1)

    return nc


def _shard_inputs(x, wq, wk, wv, wp):
    """Full inputs -> list of 8 per-core {'xlo','xhi','w'} dicts (fp16)."""
    x = np.asarray(x, dtype=np.float32)
    wq, wk, wv, wp = (np.asarray(a, dtype=np.float32) for a in (wq, wk, wv, wp))
    xf = x.reshape(B, C, N)
    kv = np.concatenate([(wk + wv).T, (wk - wv).T], axis=1)            # [64,128]
    w = np.concatenate([kv, wq.T, wp.T], axis=1).astype(np.float16)    # [64,256]
    w = np.ascontiguousarray(w)
    in_maps = []
    for core in range(8):
        bb, j = core // 4, core % 4
        xr = np.roll(xf[bb], -j * NCHUNK, axis=1).astype(np.float16)   # [64,1024]
        in_maps.append({
            "xlo": np.ascontiguousarray(xr[:, 0:512]),
            "xhi": np.ascontiguousarray(xr[:, 512:1024]),
            "w": w,
        })
    return in_maps


def _gather_outputs(results):
    """8 per-core {'out': [64,256] fp16} -> full [b,C,h,w] f32."""
    out = np.empty((B, C, N), dtype=np.float32)
    for core in range(8):
        bb, j = core // 4, core % 4
        out[bb, :, j * NCHUNK:(j + 1) * NCHUNK] = np.asarray(results[core]["out"]).astype(np.float32)
    return out.reshape(B, C, H, W)


_NC_CACHE = None


def kernel(x, wq, wk, wv, wp) -> np.ndarray:
    global _NC_CACHE
    if _NC_CACHE is None:
        _NC_CACHE = _build_nc()
    in_maps = _shard_inputs(x, wq, wk, wv, wp)
    last_err = None
    for _ in range(3):
        try:
            res = run_bass_kernel_spmd(_NC_CACHE, in_maps, core_ids=list(range(8)))
            return _gather_outputs(res.results)
        except Exception as exc:  # transient device-unrecoverable resets on retry
            last_err = exc
    raise last_err


# revision 16
# speedup vs baseline: 1.3486x; 1.0807x over previous
"""Distributed Trainium2 (Bass) kernel for nn_Attention_53764400611491.

The reference module has HEADS == C == 64, so head_dim d = C//HEADS = 1.
With d = 1 the attention algebra collapses: per (batch b, head c)

    attn = q k^T            (outer product, [N,N])
    o    = attn @ v  =  q * (k . v)        <- a scalar per (b,c)!

so the whole module is

    out[b,c,n] = sum_c' wp[c,c'] * q[b,c',n] * s[b,c'] + x[b,c,n]
    q = wq @ x_b          s[b,c'] = sum_n (wk@x_b)[c',n] * (wv@x_b)[c',n]

and the [b,h,N,N] attention tensor never needs to exist.  With
u = (wk+wv) @ x and d = (wk-wv) @ x:   s = (sum u^2 - sum d^2) / 4.

Sharding over 8 NeuronCores: core i handles batch b = i//4 and output
n-chunk j = i%4 (256 of the 1024 flattened h*w positions).  Each core
receives the full x_b (rotated so its own chunk comes first), computes
s_b redundantly, and writes its 64x256 output chunk.  No collectives:
an 8-core AllReduce has a ~10us latency floor.

v8 data path (fp16 on the wire, f32 accumulation in PSUM):
  - x ships fp16 in two [64,512] halves on the two HWDGE rings (SP +
    ACT) into one SBUF tensor.  Packed weights ride the same rings:
    [kv | wq^T wq^T] fp16 [64,256] goes FIRST on the ACT ring (so the
    uv matmuls are not gated by the slow-starting gpsimd ring), and
    the [wp^T; -wp^T] block [128,64] rides second on the SP ring (only
    needed late).  Four dynamic DMAs total; each pays ~1.5-2.5us fixed
    latency, so they all issue back-to-back at body start.
  - uv = Wkv @ x runs as two stacked fp16 matmuls; ACT squares each
    half straight out of PSUM with accum_out= (fused row-reduce).
    A dummy Square issued before any waits prefetches the PWP table.
  - q is computed REPLICATED onto 128 partitions (lhsT = [wq^T|wq^T]),
    and the final matmul contracts over 128 partitions against
    wpTs_ud = [wp^T; -wp^T] * redall * 0.25 -- this removes the
    cross-partition u/d subtract from the DVE chain entirely.
  - The residual "+x" is folded into the PSUM->SBUF eviction as DVE
    tensor_tensor adds, in two 128-column halves; each half's output
    DMA issues as soon as its half is evicted (lo on ACT, hi on SP).
  - There is NO nc.Block(): engine streams are emitted straight into
    the main body, so bass adds no end-of-body all-engine barrier and
    the compiler's teardown epilogue (which begins with its own
    all-engine sync) starts as soon as the last engine's body ends.
    The output DMAs' completion is not waited on anywhere: the
    multi-microsecond teardown (per-engine semaphore clears) covers
    the transfers, and nothing in the kernel consumes out_sem.
Measured end-to-end relative error ~8e-4 (threshold 2e-2).
"""
import numpy as np

import concourse.bass as bass
import concourse.mybir as mybir
from concourse.bass_utils import run_bass_kernel_spmd

F32 = mybir.dt.float32
F16 = mybir.dt.float16
MULT = mybir.AluOpType.mult
ADD = mybir.AluOpType.add
SQUARE = mybir.ActivationFunctionType.Square

B, C, H, W = 2, 64, 32, 32
N = H * W          # 1024
NCHUNK = N // 4    # 256 output columns per core


def _build_nc() -> bass.Bass:
    nc = bass.Bass()
    xlo_ext = nc.declare_dram_parameter("xlo", [64, 512], F16, isOutput=False)
    xhi_ext = nc.declare_dram_parameter("xhi", [64, 512], F16, isOutput=False)
    wa_ext = nc.declare_dram_parameter("wa", [64, 256], F16, isOutput=False)
    wb_ext = nc.declare_dram_parameter("wb", [128, 64], F16, isOutput=False)
    o_ext = nc.declare_dram_parameter("out", [64, 256], F16, isOutput=True)

    from contextlib import ExitStack

    with ExitStack() as ctx:
        e = ctx.enter_context
        X = e(nc.sbuf_tensor("X", [64, 1024], F16))
        Wa = e(nc.sbuf_tensor("Wa", [64, 256], F16))    # [kv | wqT wqT]
        Wb = e(nc.sbuf_tensor("Wb", [128, 64], F16))    # [wpT ; -wpT]
        sqb = e(nc.sbuf_tensor("sqb", [128, 1024], F16))  # square scratch
        redc = e(nc.sbuf_tensor("redc", [128, 2], F32))   # per-half row sums
        redall = e(nc.sbuf_tensor("redall", [128, 1], F32))
        Qsb = e(nc.sbuf_tensor("Qsb", [128, 256], F16))   # q replicated
        wpTs = e(nc.sbuf_tensor("wpTs", [128, 64], F16))
        Fsb = e(nc.sbuf_tensor("Fsb", [64, 256], F16))
        dummy = e(nc.sbuf_tensor("warmup", [1, 1], F32))
        uv1 = e(nc.psum_tensor("uv1", [128, 512], F32))
        uv2 = e(nc.psum_tensor("uv2", [128, 512], F32))
        Qp = e(nc.psum_tensor("Qp", [128, 256], F32))
        Op = e(nc.psum_tensor("Op", [64, 256], F32))
        xlo_sem = e(nc.semaphore("xlo_sem"))
        xhi_sem = e(nc.semaphore("xhi_sem"))
        wa_sem = e(nc.semaphore("wa_sem"))
        wb_sem = e(nc.semaphore("wb_sem"))
        pe_sem = e(nc.semaphore("pe_sem"))
        act_sem = e(nc.semaphore("act_sem"))
        dv_sem = e(nc.semaphore("dv_sem"))
        out_sem = e(nc.semaphore("out_sem"))

        kv = Wa[:, 0:128]
        wq2 = Wa[:, 128:256]

        sync, gp, pe, act, dv = nc.sync, nc.gpsimd, nc.tensor, nc.scalar, nc.vector

        # ---- SP (sync): x low half + wp block in, second result half out ----
        sync.dma_start(X[:, 0:512], xlo_ext[:]).then_inc(xlo_sem, 16)
        sync.dma_start(Wb[:], wb_ext[:]).then_inc(wb_sem, 16)
        sync.wait_ge(dv_sem, 4)
        # completion covered by the NEFF teardown epilogue (see header)
        sync.dma_start(o_ext[:, 128:256], Fsb[:, 128:256]).then_inc(out_sem, 16)

        # ---- PE ----
        pe.wait_ge(wa_sem, 16)
        pe.wait_ge(xlo_sem, 16)
        # rows 0-63 = u = (wk+wv)x, rows 64-127 = d = (wk-wv)x
        pe.matmul(uv1[:], kv, X[:, 0:512], start=True, stop=True).then_inc(pe_sem, 1)
        pe.wait_ge(xhi_sem, 16)
        pe.matmul(uv2[:], kv, X[:, 512:1024], start=True, stop=True).then_inc(pe_sem, 1)
        # q replicated onto both partition halves
        pe.matmul(Qp[:], wq2, X[:, 0:256], start=True, stop=True).then_inc(pe_sem, 1)
        # out = (wp diag(s)) @ q via 128-deep contraction, two column halves
        pe.wait_ge(dv_sem, 2)
        pe.matmul(Op[:, 0:128], wpTs[:], Qsb[:, 0:128], start=True, stop=True).then_inc(pe_sem, 1)
        pe.matmul(Op[:, 128:256], wpTs[:], Qsb[:, 128:256], start=True, stop=True).then_inc(pe_sem, 1)

        # ---- ACT (scalar): weights + x high half in, fused squares, first out half ----
        act.dma_start(Wa[:], wa_ext[:]).then_inc(wa_sem, 16)
        act.dma_start(X[:, 512:1024], xhi_ext[:]).then_inc(xhi_sem, 16)
        # warm the ACT Square table while DMAs are in flight
        act.activation(dummy[:], nc.const_aps.tensor(0.0, (1, 1), F32), SQUARE).then_inc(act_sem, 1)
        act.wait_ge(pe_sem, 1)
        act.activation(sqb[:, 0:512], uv1[:], SQUARE, accum_out=redc[:, 0:1]).then_inc(act_sem, 1)
        act.wait_ge(pe_sem, 2)
        act.activation(sqb[:, 512:1024], uv2[:], SQUARE, accum_out=redc[:, 1:2]).then_inc(act_sem, 1)
        act.wait_ge(dv_sem, 3)
        act.dma_start(o_ext[:, 0:128], Fsb[:, 0:128]).then_inc(out_sem, 16)

        # ---- DVE (vector) ----
        # q PSUM -> SBUF fp16 in DVE's idle window
        dv.wait_ge(pe_sem, 3)
        dv.tensor_copy(Qsb[:], Qp[:]).then_inc(dv_sem, 1)
        dv.wait_ge(act_sem, 3)
        dv.wait_ge(wb_sem, 16)
        dv.reduce_sum(redall[:], redc[:], axis=mybir.AxisListType.X)
        dv.drain()  # redall landed (same-engine RAW)
        # wpTs = [wpT; -wpT] * (su; sd) * 0.25  (the u/d subtract happens
        # inside the final matmul via the negated lower half)
        dv.tensor_scalar(wpTs[:], Wb[:], redall[:], 0.25, op0=MULT, op1=MULT).then_inc(dv_sem, 1)
        dv.wait_ge(pe_sem, 4)
        # evict out halves PSUM -> SBUF with the "+ x" residual folded in
        dv.tensor_tensor(Fsb[:, 0:128], Op[:, 0:128], X[:, 0:128], ADD).then_inc(dv_sem, 1)
        dv.wait_ge(pe_sem, 5)
        dv.tensor_tensor(Fsb[:, 128:256], Op[:, 128:256], X[:, 128:256], ADD).then_inc(dv_sem, 1)

    return nc


def _shard_inputs(x, wq, wk, wv, wp):
    """Full inputs -> list of 8 per-core {'xlo','xhi','wa','wb'} dicts (fp16)."""
    x = np.asarray(x, dtype=np.float32)
    wq, wk, wv, wp = (np.asarray(a, dtype=np.float32) for a in (wq, wk, wv, wp))
    xf = x.reshape(B, C, N)
    kv = np.concatenate([(wk + wv).T, (wk - wv).T], axis=1)                  # [64,128]
    wa = np.concatenate([kv, wq.T, wq.T], axis=1).astype(np.float16)         # [64,256]
    wa = np.ascontiguousarray(wa)
    wb = np.concatenate([wp.T, -wp.T], axis=0).astype(np.float16)            # [128,64]
    wb = np.ascontiguousarray(wb)
    in_maps = []
    for core in range(8):
        bb, j = core // 4, core % 4
        xr = np.roll(xf[bb], -j * NCHUNK, axis=1).astype(np.float16)         # [64,1024]
        in_maps.append({
            "xlo": np.ascontiguousarray(xr[:, 0:512]),
            "xhi": np.ascontiguousarray(xr[:, 512:1024]),
            "wa": wa,
            "wb": wb,
        })
    return in_maps


def _gather_outputs(results):
    """8 per-core {'out': [64,256] fp16} -> full [b,C,h,w] f32."""
    out = np.empty((B, C, N), dtype=np.float32)
    for core in range(8):
        bb, j = core // 4, core % 4
        out[bb, :, j * NCHUNK:(j + 1) * NCHUNK] = np.asarray(results[core]["out"]).astype(np.float32)
    return out.reshape(B, C, H, W)


_NC_CACHE = None


def kernel(x, wq, wk, wv, wp) -> np.ndarray:
    global _NC_CACHE
    if _NC_CACHE is None:
        _NC_CACHE = _build_nc()
    in_maps = _shard_inputs(x, wq, wk, wv, wp)
    last_err = None
    for _ in range(3):
        try:
            res = run_bass_kernel_spmd(_NC_CACHE, in_maps, core_ids=list(range(8)))
            return _gather_outputs(res.results)
        except Exception as exc:  # transient device-unrecoverable resets on retry
            last_err = exc
    raise last_err
